# revision 56
# baseline (speedup 1.0000x reference)
"""Multi-head causal self-attention on 8 Trainium2 NeuronCores.

Problem: x[4,2048,1024] fp32, Wq/Wk/Wv/Wo[1024,1024], H=16 heads, head_dim=64,
causal mask, attention_mask all-ones (per spec fill=ones -> no-op).

Sharding (hybrid data/tensor parallel):
  core c -> batch b = c//2, head-half hh = c%2 (8 heads = 512 features).
  Each core: Q/K/V projections with column-sliced W (Megatron column
  parallel), attention for its 8 heads, o_proj with row-sliced Wo
  (row parallel) producing a partial [2048,1024] output. The host sums
  the two partials per batch (the "all-reduce") and stacks batches.

Device kernel (per core), all matmuls bf16 in / fp32 PSUM accumulate:
  QT/KT produced in transposed [feat, seq] layout directly (x is fed
  pre-transposed from the host), scores are computed transposed
  (scoresT[k,q] = KT_blk.T @ QT) so softmax needs no on-chip transpose:
  exp runs on ScalarE straight out of PSUM.  AV runs in the transposed
  orientation: out[q, hd] accumulates lhsT=expT[k, q-block] x rhs=V[k, hd]
  per 128-wide q-block, so each accumulation step is charged only 65
  output columns (64 v-dims + a fused ones-column giving sumexp) instead
  of 512 -- half the PE work of the untransposed form.  Normalization is
  then a per-partition scalar multiply (DVE tensor_scalar) into a bf16
  staging tile, and the [q, feat] -> [feat, q] layout flip that o_proj
  needs is done by the DMA crossbar (dma_start_transpose), which costs
  zero PE/DVE time.  Causal handling: block-skip fully-masked key blocks,
  one 128x128 triangular mask multiply on diagonal blocks.
"""

import numpy as np
import ml_dtypes

_BF16 = ml_dtypes.bfloat16
_B, _S, _D = 4, 2048, 1024
_NCORES = 8
_HPC = 8   # heads per core
_FT = 4    # 128-wide feature tiles per core (= head pairs)
_DT = 8    # 128-wide tiles of D
_SB = 16   # 128-wide seq blocks
_QC = 4    # 512-wide seq chunks

_cache = {}


def _build_nc(opts=None):
    opts = opts or {}
    import concourse.bacc as bacc
    import concourse.mybir as mybir
    import concourse.tile as tile
    from concourse.bass import ts

    f32 = mybir.dt.float32
    bf16 = mybir.dt.bfloat16
    Exp = mybir.ActivationFunctionType.Exp

    nc = bacc.Bacc("TRN2", target_bir_lowering=False, debug=False)

    xt = nc.dram_tensor("xt", [_D, _S], bf16, kind="ExternalInput")   # x[b].T
    wq = nc.dram_tensor("wq", [_D, 512], bf16, kind="ExternalInput")  # pre-scaled 1/8
    wk = nc.dram_tensor("wk", [_D, 512], bf16, kind="ExternalInput")
    wv = nc.dram_tensor("wv", [_D, 512], bf16, kind="ExternalInput")
    wo = nc.dram_tensor("wo", [512, _D], bf16, kind="ExternalInput")
    # y partials in bf16: host accumulates in fp32; halves output DMA time
    y = nc.dram_tensor("y", [_S, _D], bf16, kind="ExternalOutput")

    with tile.TileContext(nc) as tc:
        with (
            tc.tile_pool(name="const", bufs=1) as constp,
            tc.tile_pool(name="win", bufs=1) as wp,
            tc.tile_pool(name="acts", bufs=1) as actp,
            tc.tile_pool(name="ex", bufs=opts.get("ex_bufs", 18)) as exp_pool,
            tc.tile_pool(name="ev", bufs=opts.get("ev_bufs", 6)) as ev_pool,
            tc.tile_pool(name="stg", bufs=opts.get("stg_bufs", 2)) as stg_pool,
            tc.tile_pool(name="nrm", bufs=opts.get("nrm_bufs", 4)) as nrm_pool,
            tc.tile_pool(name="ps_proj", bufs=opts.get("proj_bufs", 2), space="PSUM") as ps_proj,
            tc.tile_pool(name="ps_big", bufs=2, space="PSUM") as ps_big,
            tc.tile_pool(name="ps_av", bufs=opts.get("av_bufs", 2), space="PSUM") as ps_av,
        ):
            # ---- input loads, split and ordered by first use so the first
            # matmuls start after ~1/8 of the bytes land ---------------------
            xts = wp.tile([128, _DT, _S], bf16, name="xts", tag="xts")
            wqs = wp.tile([128, _DT, 512], bf16, name="wqs", tag="wqs")
            wks = wp.tile([128, _DT, 512], bf16, name="wks", tag="wks")
            wvs = wp.tile([128, _DT, 512], bf16, name="wvs", tag="wvs")
            wos = wp.tile([128, _FT, _D], bf16, name="wos", tag="wos")
            xt_r = xt[:].rearrange("(dt p) s -> p dt s", p=128)
            wq_r = wq[:].rearrange("(dt p) n -> p dt n", p=128)
            wk_r = wk[:].rearrange("(dt p) n -> p dt n", p=128)
            wv_r = wv[:].rearrange("(dt p) n -> p dt n", p=128)
            # three DGE paths in parallel: SP streams wv (+late xt chunks),
            # Act streams xt chunk 0, gpsimd's software DGE streams wq/wk --
            # the 650ns/issue sequencer serialization stops gating startup
            dm = opts.get("dma_mode", 0)
            if dm == 3:
                # wq/wk dh0 woven between the wv/xt pieces so the first
                # q/k projection chains start ~3us earlier
                for dts in (slice(0, 1), slice(1, 2)):
                    nc.sync.dma_start(wvs[:, dts], wv_r[:, dts])
                    nc.sync.dma_start(xts[:, dts, 0:512], xt_r[:, dts, 0:512])
                nc.sync.dma_start(wqs[:, 0:4], wq_r[:, 0:4])
                nc.sync.dma_start(wvs[:, 2:4], wv_r[:, 2:4])
                nc.sync.dma_start(xts[:, 2:4, 0:512], xt_r[:, 2:4, 0:512])
                nc.sync.dma_start(wks[:, 0:4], wk_r[:, 0:4])
                nc.sync.dma_start(wvs[:, 4:8], wv_r[:, 4:8])
                nc.sync.dma_start(xts[:, 4:8, 0:512], xt_r[:, 4:8, 0:512])
                nc.sync.dma_start(wqs[:, 4:8], wq_r[:, 4:8])
                nc.sync.dma_start(wks[:, 4:8], wk_r[:, 4:8])
            else:
                xt_eng = nc.scalar if dm == 1 else nc.sync
                qk_eng = nc.scalar if dm == 4 else (nc.gpsimd if dm == 2 else nc.sync)
                for dts in (slice(0, 1), slice(1, 2), slice(2, 4), slice(4, 8)):
                    nc.sync.dma_start(wvs[:, dts], wv_r[:, dts])
                    xt_eng.dma_start(xts[:, dts, 0:512], xt_r[:, dts, 0:512])
                for dh in range(2):
                    dts = slice(dh * 4, dh * 4 + 4)
                    qk_eng.dma_start(wqs[:, dts], wq_r[:, dts])
                    qk_eng.dma_start(wks[:, dts], wk_r[:, dts])
            for sc in range(1, _QC):
                nc.sync.dma_start(
                    xts[:, :, ts(sc, 512)], xt_r[:, :, ts(sc, 512)])
            nc.sync.dma_start(wos[:], wo[:].rearrange("(ft p) n -> p ft n", p=128))

            # causal mask for diagonal 128x128 blocks: keep iff q_rel >= k_rel
            # (one copy per head of the pair so a single strided multiply
            # masks both heads' diagonal blocks)
            mask0 = constp.tile([128, 128], bf16, name="mask0", tag="mask0")
            nc.gpsimd.memset(mask0[:], 1.0)
            nc.gpsimd.affine_select(
                out=mask0[:], in_=mask0[:],
                compare_op=mybir.AluOpType.is_ge, fill=0.0,
                base=0, channel_multiplier=-1, pattern=[[1, 128]],
            )
            mask2 = constp.tile([128, 2, 128], bf16, name="mask2", tag="mask2")
            for _h in range(2):
                nc.gpsimd.tensor_copy(mask2[:, _h, :], mask0[:])

            # V with a ones-column appended per head: [128, sb, head, 64+1]
            vxs = actp.tile([128, _SB, _HPC, 65], bf16, name="vxs", tag="vxs")
            nc.gpsimd.memset(vxs[:, :, :, 64], 1.0)

            qts = [actp.tile([128, _S], bf16, name=f"qt{ft}", tag=f"qt{ft}") for ft in range(_FT)]
            kts = [actp.tile([128, _S], bf16, name=f"kt{ft}", tag=f"kt{ft}") for ft in range(_FT)]
            # ctx as one tile [128, ft, S] so dma-transpose writes all 4
            # feature tiles of a q-block in one instruction
            ctxm = actp.tile([128, _FT, _S], bf16, name="ctxm", tag="ctxm")

            proj_ctr = [0]

            def proj_cp(out_ap, in_ap):
                # alternate psum->sbuf proj copies between DVE and the (early
                # idle) Act engine so the psproj rotation turns around faster
                proj_ctr[0] += 1
                if (opts.get("proj_cp_alt", False)
                        and proj_ctr[0] <= opts.get("proj_cp_n", 12)
                        and proj_ctr[0] % 2):
                    nc.scalar.copy(out_ap, in_ap)
                else:
                    nc.vector.tensor_copy(out_ap, in_ap)

            def proj_v(sb):
                ps = ps_proj.tile([128, 512], f32, tag="psproj", name="psv")
                for dt in range(_DT):
                    nc.tensor.matmul(
                        ps[:], lhsT=xts[:, dt, ts(sb, 128)], rhs=wvs[:, dt, :],
                        start=(dt == 0), stop=(dt == _DT - 1),
                    )
                proj_cp(
                    vxs[:, sb, :, 0:64], ps[:].rearrange("p (h d) -> p h d", h=_HPC)
                )

            def proj_qk1(sc, ft, which):
                wsrc, dst = ((wqs, qts[ft]) if which == 0 else (wks, kts[ft]))
                ps = ps_proj.tile([128, 512], f32, tag="psproj", name="psqk")
                for dt in range(_DT):
                    nc.tensor.matmul(
                        ps[:], lhsT=wsrc[:, dt, ts(ft, 128)],
                        rhs=xts[:, dt, ts(sc, 512)],
                        start=(dt == 0), stop=(dt == _DT - 1),
                    )
                proj_cp(dst[:, ts(sc, 512)], ps[:])

            def chunk_tasks(sc):
                # projection work needed before attention chunk sc runs
                t = [(lambda sb=sb: proj_v(sb)) for sb in range(4 * sc, 4 * sc + 4)]
                t += [(lambda sc=sc, ft=ft, w=w: proj_qk1(sc, ft, w))
                      for ft in range(_FT) for w in range(2)]
                return t

            for t in chunk_tasks(0):
                t()

            def oproj_qb(qb, pools=None, flush=False, last=False):
                # full output row-block qb: two 512-wide psum halves share
                # one ev tile and a single y DMA (halves software-DGE count).
                # The very last block issues per-half DMAs so the first half
                # drains while the second is still in o_proj.
                ev = ev_pool.tile([128, 1024], bf16, tag="ev2", name="ev2")
                for nn2 in range(2):
                    pool, tag = (pools[nn2] if pools else (ps_proj, "psproj"))
                    ps = pool.tile([128, 512], f32, tag=tag, name="pso")
                    for ft in range(_FT):
                        nc.tensor.matmul(
                            ps[:], lhsT=ctxm[:, ft, ts(qb, 128)],
                            rhs=wos[:, ft, ts(nn2, 512)],
                            start=(ft == 0), stop=(ft == _FT - 1),
                        )
                    cp = (nc.scalar.copy if (opts.get("ev_alt", True) and flush and nn2)
                          else nc.vector.tensor_copy)
                    cp(ev[:, ts(nn2, 512)], ps[:])
                    if last:
                        eng = nc.sync if nn2 else nc.gpsimd
                        eng.dma_start(
                            y[:][ts(qb, 128), ts(nn2, 512)], ev[:, ts(nn2, 512)])
                if not last:
                    # y writes ride the gpsimd software DGE so they never
                    # queue behind a dma-transpose waiting on SP.SEQ
                    eng = (nc.sync if (opts.get("y_alt", True) and flush and qb % 2)
                           else nc.gpsimd)
                    eng.dma_start(y[:][ts(qb, 128), :], ev[:])

            def emit_scores(qc, hp, kbg):
                # scoresT[k, q] = KT_blk.T @ QT_chunk; heads of the pair
                # interleave (row groups 0-63 / 64-127 run concurrently).
                # Non-diagonal kb-groups: h01-major [128, 1024] tiles, one
                # exp per head covering both key blocks. Diagonal groups:
                # kb-major [128, 2, 512] tiles so one 3D-AP exp per key
                # block covers both heads despite the causal column trim.
                soffs = []
                for kbl in range(2):
                    kb = 2 * kbg + kbl
                    off = (kb - 4 * qc) * 128
                    soffs.append(off if off > 0 else 0)
                kb_major = soffs[1] > 0
                scps = [ps_big.tile([128, 2, 512] if kb_major else [128, 1024],
                                    f32, name=f"scp{_i}", tag="scp")
                        for _i in range(2)]
                exs = [exp_pool.tile([128, 2, 512] if kb_major else [128, 1024],
                                     bf16, name=f"ex{_i}", tag="ex")
                       for _i in range(2)]
                # matmul order matches act coverage: h01-major tiles pair
                # (h01, both kbls), kb-major tiles pair (kbl, both h01s) --
                # either way the first act starts after two matmuls
                order = ([(h01, kbl) for kbl in range(2) for h01 in range(2)]
                         if kb_major else
                         [(h01, kbl) for h01 in range(2) for kbl in range(2)])
                for h01, kbl in order:
                    pb = 64 * h01
                    kb = 2 * kbg + kbl
                    soff = soffs[kbl]
                    dst = (scps[kbl][:, h01, soff:512] if kb_major
                           else scps[h01][:, kbl * 512 + soff:(kbl + 1) * 512])
                    nc.tensor.matmul(
                        dst,
                        lhsT=kts[hp][pb:pb + 64, ts(kb, 128)],
                        rhs=qts[hp][pb:pb + 64, qc * 512 + soff:(qc + 1) * 512],
                        start=True, stop=True,
                    )
                if kb_major:
                    for kbl in range(2):
                        soff = soffs[kbl]
                        nc.scalar.activation(
                            exs[kbl][:, :, soff:512], scps[kbl][:, :, soff:512], Exp)
                else:
                    for h01 in range(2):
                        nc.scalar.activation(
                            exs[h01][:, soffs[0]:1024],
                            scps[h01][:, soffs[0]:1024], Exp)
                return (kb_major, exs)

            def emit_av(qc, hp, kbg, ex_info, avs):
                # diagonal masks first, then the AV matmuls in transposed
                # orientation: avs[h01][q-part, qb, 65] accumulates
                # lhsT=expT[k, q-block] x rhs=V[k, 65] over key blocks.
                kb_major, exs = ex_info
                maskeng = nc.gpsimd if opts.get("mask_gpsimd") else nc.vector
                for kbl in range(2):
                    kb = 2 * kbg + kbl
                    off = (kb - 4 * qc) * 128
                    if off < 0:
                        continue
                    if kb_major:
                        if opts.get("mask_split"):
                            for h01 in range(2):
                                maskeng.tensor_mul(
                                    exs[kbl][:, h01, off:off + 128],
                                    exs[kbl][:, h01, off:off + 128],
                                    mask0[:],
                                )
                        else:
                            maskeng.tensor_mul(
                                exs[kbl][:, :, off:off + 128],
                                exs[kbl][:, :, off:off + 128],
                                mask2[:],
                            )
                    else:
                        for h01 in range(2):
                            maskeng.tensor_mul(
                                exs[h01][:, kbl * 512 + off:kbl * 512 + off + 128],
                                exs[h01][:, kbl * 512 + off:kbl * 512 + off + 128],
                                mask0[:],
                            )
                # psum "zero regions" are bank-granular: exactly ONE
                # start (first matmul into the tile: it marks the whole
                # bank pending-zero, so each chain's first write zeroes
                # its own bytes) and ONE stop (very last matmul).
                ao = opts.get("av_order", 0)
                if ao == 0:
                    order = [(h01, kbl, qb) for h01 in range(2)
                             for kbl in range(2) for qb in range(4)]
                elif ao == 1:
                    order = [(h01, kbl, qb) for kbl in range(2)
                             for h01 in range(2) for qb in range(4)]
                else:
                    order = [(h01, kbl, qb) for kbl in range(2)
                             for qb in range(4) for h01 in range(2)]
                for h01, kbl, qb in order:
                    kb = 2 * kbg + kbl
                    off = (kb - 4 * qc) * 128
                    h = 2 * hp + h01
                    if qb * 128 < off:
                        continue  # block fully above the diagonal
                    lhsT = (exs[kbl][:, h01, qb * 128:(qb + 1) * 128]
                            if kb_major else
                            exs[h01][:, kbl * 512 + qb * 128:kbl * 512 + (qb + 1) * 128])
                    nc.tensor.matmul(
                        avs[h01][:, qb, :],
                        lhsT=lhsT,
                        rhs=vxs[:, kb, h, :],
                        start=(kb == 0 and qb == 0),
                        stop=(kb == 4 * qc + 3 and qb == 3),
                    )

            def emit_norm_qbs(qc, hp, avs, stage, qbs):
                # norm a pair of q-blocks as soon as their AV chains stop
                # (used on the very last pass so transposes/o_proj overlap
                # the exp drain instead of serializing after it)
                for h01 in range(2):
                    rc = nrm_pool.tile([128, 2], f32, tag="rc2")
                    nc.vector.reciprocal(rc[:], avs[h01][:, qbs[0]:qbs[0] + 2, 64])
                    for i, qb in enumerate(qbs):
                        nc.vector.tensor_scalar_mul(
                            stage[:, qb, hp * 128 + h01 * 64:hp * 128 + h01 * 64 + 64],
                            avs[h01][:, qb, 0:64],
                            rc[:, i:i + 1],
                        )

            def emit_transpose_qb(qc, stage, qb):
                nc.sync.dma_start_transpose(
                    ctxm[:, :, qc * 512 + qb * 128:qc * 512 + (qb + 1) * 128],
                    stage[:, qb, :],
                )

            def emit_norm(qc, hp, avs, stage):
                # ctx[q, f] = avU[q, f] / sumexp[q]: per-partition scalar
                # multiply by the reciprocal of the fused ones-column.
                from concourse.bass import broadcast_tensor_aps
                for h01 in range(2):
                    rc = nrm_pool.tile([128, 4], f32, tag="rc")
                    nc.vector.reciprocal(rc[:], avs[h01][:, :, 64])
                    eng = (nc.gpsimd if (opts.get("norm_split") and h01)
                           else nc.vector)
                    if opts.get("norm_bcast", False):
                        # one mul for all 4 q-blocks: rc broadcast along the
                        # feature dim with a stride-0 AP
                        out_ap = stage[:, :, hp * 128 + h01 * 64:hp * 128 + h01 * 64 + 64]
                        in0 = avs[h01][:, :, 0:64]
                        in1 = rc[:].rearrange("p (q o) -> p q o", o=1)
                        in0b, in1b = broadcast_tensor_aps(in0, in1)
                        eng.tensor_mul(out_ap, in0b, in1b)
                    else:
                        for qb in range(4):
                            eng.tensor_scalar_mul(
                                stage[:, qb, hp * 128 + h01 * 64:hp * 128 + h01 * 64 + 64],
                                avs[h01][:, qb, 0:64],
                                rc[:, qb:qb + 1],
                            )

            def emit_transpose(qc, stage):
                # [q, f] -> [f, q] on the DMA crossbar: zero PE/DVE cost.
                # Last chunk alternates SP/Act issue so the four transposes
                # don't serialize on one sequencer right before the flush.
                for qb in range(4):
                    eng = (nc.scalar if (opts.get("tp_alt") and qc == _QC - 1 and qb % 2)
                           else nc.sync)
                    eng.dma_start_transpose(
                        ctxm[:, :, qc * 512 + qb * 128:qc * 512 + (qb + 1) * 128],
                        stage[:, qb, :],
                    )

            pending = []
            soft = []  # o_proj work: deferred to the end flush (measured fastest)

            def finish_unit(qc, hp, kbg, ex_info):
                key = (qc, hp)
                avs = avs_by_hp.get(key)
                if avs is None:
                    avs = [ps_av.tile([128, 4, 65], f32, name=f"av{_i}", tag="av")
                           for _i in range(2)]
                    avs_by_hp[key] = avs
                emit_av(qc, hp, kbg, ex_info, avs)
                last_pass = (qc == _QC - 1 and hp == _FT - 1
                             and opts.get("early_tail", False))
                stage = stage_by_qc.get(qc)
                if stage is None:
                    stage = stg_pool.tile([128, 4, 512], bf16, tag="stage")
                    stage_by_qc[qc] = stage
                if last_pass and kbg == 2 * qc:
                    emit_norm_qbs(qc, hp, avs, stage, (0, 1))
                    emit_transpose_qb(qc, stage, 0)
                    emit_transpose_qb(qc, stage, 1)
                if kbg == 2 * qc + 1:
                    avs_by_hp.pop(key)
                    if last_pass:
                        emit_norm_qbs(qc, hp, avs, stage, (2, 3))
                        emit_transpose_qb(qc, stage, 2)
                        emit_transpose_qb(qc, stage, 3)
                        stage_by_qc.pop(qc)
                        flush_final(list(range(4 * qc, 4 * qc + 4)))
                    else:
                        emit_norm(qc, hp, avs, stage)
                        if hp == _FT - 1:
                            emit_transpose(qc, stage_by_qc.pop(qc))
                            ot = list(range(4 * qc, 4 * qc + 4))
                            if qc + 1 < _QC:
                                soft.extend(ot)
                            else:
                                flush_final(ot)

            def flush_final(otasks):
                # final flush: rotate the finished chunk's o_proj
                # groups across all freed psum pools
                rot = [(ps_proj, "psproj"), (ps_big, "scp"), (ps_av, "av")]
                for qb in soft:
                    oproj_qb(qb, flush=True)
                for i, qb in enumerate(otasks):
                    pools = (rot[(2 * i) % 3], rot[(2 * i + 1) % 3])
                    oproj_qb(qb, pools, flush=True,
                             last=(opts.get("tail_split", True) and i == len(otasks) - 1))

            # software-pipeline the emission across hp AND qc
            # boundaries: scores of the next kb-group enter the PE
            # stream before AV of the previous, so PE never stalls
            # on exp latency.
            inflight = []
            avs_by_hp = {}
            stage_by_qc = {}
            kb_ctr = 0
            for qc in range(_QC):
                pending.extend(chunk_tasks(qc + 1) if qc + 1 < _QC else [])
                kbs_left = _FT * (2 * qc + 2)
                for hp in range(_FT):
                    for kbg in range(2 * qc + 2):
                        # interleave next chunk's projection work
                        npop = -(-len(pending) // kbs_left) if pending else 0
                        kbs_left -= 1
                        for _ in range(npop):
                            pending.pop(0)()
                        drip_ok = (qc == _QC - 1) if opts.get("drip_qc3") else True
                        if (soft and not pending and drip_ok
                                and kb_ctr % opts.get("soft_rate", 8) == 0):
                            oproj_qb(soft.pop(0))
                        kb_ctr += 1
                        inflight.append((qc, hp, kbg, emit_scores(qc, hp, kbg)))
                        if len(inflight) >= opts.get("depth", 9):
                            q0, h0, k0, e0 = inflight.pop(0)
                            finish_unit(q0, h0, k0, e0)
            for q0, h0, k0, e0 in inflight:
                finish_unit(q0, h0, k0, e0)

            if opts.get("dbg"):
                ydbg = nc.dram_tensor("ctx_dbg", [128, _FT, _S], bf16,
                                      kind="ExternalOutput")
                nc.sync.dma_start(ydbg[:], ctxm[:])
                qdbg = nc.dram_tensor("q_dbg", [128, _FT, _S], bf16,
                                      kind="ExternalOutput")
                for ft in range(_FT):
                    nc.sync.dma_start(qdbg[:][:, ft, :], qts[ft][:])
                kdbg = nc.dram_tensor("k_dbg", [128, _FT, _S], bf16,
                                      kind="ExternalOutput")
                for ft in range(_FT):
                    nc.sync.dma_start(kdbg[:][:, ft, :], kts[ft][:])
                vdbg = nc.dram_tensor("v_dbg", [128, _SB, _HPC, 65], bf16,
                                      kind="ExternalOutput")
                nc.sync.dma_start(vdbg[:], vxs[:])

    nc.compile()
    return nc


def _get_nc(opts=None):
    key = tuple(sorted((opts or {}).items()))
    if key not in _cache:
        _cache[key] = _build_nc(opts)
    return _cache[key]


def _shard(x, Wq, Wk, Wv, Wo):
    in_maps = []
    for c in range(_NCORES):
        b, hh = divmod(c, 2)
        cols = slice(512 * hh, 512 * hh + 512)
        in_maps.append({
            "xt": np.ascontiguousarray(x[b].T).astype(_BF16),
            "wq": (Wq[:, cols] * np.float32(0.125)).astype(_BF16),
            "wk": np.ascontiguousarray(Wk[:, cols]).astype(_BF16),
            "wv": np.ascontiguousarray(Wv[:, cols]).astype(_BF16),
            "wo": np.ascontiguousarray(Wo[cols, :]).astype(_BF16),
        })
    return in_maps


def _run(inputs, trace=False):
    from concourse import bass_utils

    x = np.asarray(inputs["x"], dtype=np.float32)
    Wq = np.asarray(inputs["Wq"], dtype=np.float32)
    Wk = np.asarray(inputs["Wk"], dtype=np.float32)
    Wv = np.asarray(inputs["Wv"], dtype=np.float32)
    Wo = np.asarray(inputs["Wo"], dtype=np.float32)
    # attention_mask is all-ones by problem spec (fill=ones) -> no-op.

    nc = _get_nc()
    res = bass_utils.run_bass_kernel_spmd(
        nc, _shard(x, Wq, Wk, Wv, Wo), core_ids=list(range(_NCORES)), trace=trace
    )
    ys = [np.asarray(r["y"]).astype(np.float32) for r in res.results]
    out = np.stack([ys[2 * b] + ys[2 * b + 1] for b in range(_B)])
    return out, res


def kernel(**inputs):
    return _run(inputs)[0]


# revision 59
# speedup vs baseline: 1.0027x; 1.0027x over previous
"""Multi-head causal self-attention on 8 Trainium2 NeuronCores.

Problem: x[4,2048,1024] fp32, Wq/Wk/Wv/Wo[1024,1024], H=16 heads, head_dim=64,
causal mask, attention_mask all-ones (per spec fill=ones -> no-op).

Sharding (hybrid data/tensor parallel):
  core c -> batch b = c//2, head-half hh = c%2 (8 heads = 512 features).
  Each core: Q/K/V projections with column-sliced W (Megatron column
  parallel), attention for its 8 heads, o_proj with row-sliced Wo
  (row parallel) producing a partial [2048,1024] output. The host sums
  the two partials per batch (the "all-reduce") and stacks batches.

Device kernel (per core), all matmuls bf16 in / fp32 PSUM accumulate:
  QT/KT produced in transposed [feat, seq] layout directly (x is fed
  pre-transposed from the host), scores are computed transposed
  (scoresT[k,q] = KT_blk.T @ QT) so softmax needs no on-chip transpose:
  exp runs on ScalarE straight out of PSUM.  AV runs in the transposed
  orientation: out[q, hd] accumulates lhsT=expT[k, q-block] x rhs=V[k, hd]
  per 128-wide q-block, so each accumulation step is charged only 65
  output columns (64 v-dims + a fused ones-column giving sumexp) instead
  of 512 -- half the PE work of the untransposed form.  Normalization is
  then a per-partition scalar multiply (DVE tensor_scalar) into a bf16
  staging tile, and the [q, feat] -> [feat, q] layout flip that o_proj
  needs is done by the DMA crossbar (dma_start_transpose), which costs
  zero PE/DVE time.  Causal handling: block-skip fully-masked key blocks,
  one 128x128 triangular mask multiply on diagonal blocks.
"""

import numpy as np
import ml_dtypes

_BF16 = ml_dtypes.bfloat16
_B, _S, _D = 4, 2048, 1024
_NCORES = 8
_HPC = 8   # heads per core
_FT = 4    # 128-wide feature tiles per core (= head pairs)
_DT = 8    # 128-wide tiles of D
_SB = 16   # 128-wide seq blocks
_QC = 4    # 512-wide seq chunks

_cache = {}


def _build_nc(opts=None):
    opts = opts or {}
    import concourse.bacc as bacc
    import concourse.mybir as mybir
    import concourse.tile as tile
    from concourse.bass import ts

    f32 = mybir.dt.float32
    bf16 = mybir.dt.bfloat16
    Exp = mybir.ActivationFunctionType.Exp

    nc = bacc.Bacc("TRN2", target_bir_lowering=False, debug=False)

    xt = nc.dram_tensor("xt", [_D, _S], bf16, kind="ExternalInput")   # x[b].T
    wq = nc.dram_tensor("wq", [_D, 512], bf16, kind="ExternalInput")  # pre-scaled 1/8
    wk = nc.dram_tensor("wk", [_D, 512], bf16, kind="ExternalInput")
    wv = nc.dram_tensor("wv", [_D, 512], bf16, kind="ExternalInput")
    wo = nc.dram_tensor("wo", [512, _D], bf16, kind="ExternalInput")
    # y partials in bf16: host accumulates in fp32; halves output DMA time
    y = nc.dram_tensor("y", [_S, _D], bf16, kind="ExternalOutput")

    with tile.TileContext(nc) as tc:
        with (
            tc.tile_pool(name="const", bufs=1) as constp,
            tc.tile_pool(name="win", bufs=1) as wp,
            tc.tile_pool(name="acts", bufs=1) as actp,
            tc.tile_pool(name="ex", bufs=opts.get("ex_bufs", 18)) as exp_pool,
            tc.tile_pool(name="ev", bufs=opts.get("ev_bufs", 6)) as ev_pool,
            tc.tile_pool(name="stg", bufs=opts.get("stg_bufs", 2)) as stg_pool,
            tc.tile_pool(name="nrm", bufs=opts.get("nrm_bufs", 4)) as nrm_pool,
            tc.tile_pool(name="ps_proj", bufs=opts.get("proj_bufs", 2), space="PSUM") as ps_proj,
            tc.tile_pool(name="ps_big", bufs=2, space="PSUM") as ps_big,
            tc.tile_pool(name="ps_av", bufs=opts.get("av_bufs", 2), space="PSUM") as ps_av,
        ):
            # ---- input loads, split and ordered by first use so the first
            # matmuls start after ~1/8 of the bytes land ---------------------
            xts = wp.tile([128, _DT, _S], bf16, name="xts", tag="xts")
            wqs = wp.tile([128, _DT, 512], bf16, name="wqs", tag="wqs")
            wks = wp.tile([128, _DT, 512], bf16, name="wks", tag="wks")
            wvs = wp.tile([128, _DT, 512], bf16, name="wvs", tag="wvs")
            wos = wp.tile([128, _FT, _D], bf16, name="wos", tag="wos")
            xt_r = xt[:].rearrange("(dt p) s -> p dt s", p=128)
            wq_r = wq[:].rearrange("(dt p) n -> p dt n", p=128)
            wk_r = wk[:].rearrange("(dt p) n -> p dt n", p=128)
            wv_r = wv[:].rearrange("(dt p) n -> p dt n", p=128)
            # three DGE paths in parallel: SP streams wv (+late xt chunks),
            # Act streams xt chunk 0, gpsimd's software DGE streams wq/wk --
            # the 650ns/issue sequencer serialization stops gating startup
            dm = opts.get("dma_mode", 0)
            if dm == 3:
                # wq/wk dh0 woven between the wv/xt pieces so the first
                # q/k projection chains start ~3us earlier
                for dts in (slice(0, 1), slice(1, 2)):
                    nc.sync.dma_start(wvs[:, dts], wv_r[:, dts])
                    nc.sync.dma_start(xts[:, dts, 0:512], xt_r[:, dts, 0:512])
                nc.sync.dma_start(wqs[:, 0:4], wq_r[:, 0:4])
                nc.sync.dma_start(wvs[:, 2:4], wv_r[:, 2:4])
                nc.sync.dma_start(xts[:, 2:4, 0:512], xt_r[:, 2:4, 0:512])
                nc.sync.dma_start(wks[:, 0:4], wk_r[:, 0:4])
                nc.sync.dma_start(wvs[:, 4:8], wv_r[:, 4:8])
                nc.sync.dma_start(xts[:, 4:8, 0:512], xt_r[:, 4:8, 0:512])
                nc.sync.dma_start(wqs[:, 4:8], wq_r[:, 4:8])
                nc.sync.dma_start(wks[:, 4:8], wk_r[:, 4:8])
            else:
                xt_eng = nc.scalar if dm == 1 else nc.sync
                qk_eng = nc.scalar if dm == 4 else (nc.gpsimd if dm == 2 else nc.sync)
                for dts in (slice(0, 1), slice(1, 2), slice(2, 4), slice(4, 8)):
                    nc.sync.dma_start(wvs[:, dts], wv_r[:, dts])
                    xt_eng.dma_start(xts[:, dts, 0:512], xt_r[:, dts, 0:512])
                for dh in range(2):
                    dts = slice(dh * 4, dh * 4 + 4)
                    qk_eng.dma_start(wqs[:, dts], wq_r[:, dts])
                    qk_eng.dma_start(wks[:, dts], wk_r[:, dts])
            for sc in range(1, _QC):
                nc.sync.dma_start(
                    xts[:, :, ts(sc, 512)], xt_r[:, :, ts(sc, 512)])
            nc.sync.dma_start(wos[:], wo[:].rearrange("(ft p) n -> p ft n", p=128))

            # causal mask for diagonal 128x128 blocks: keep iff q_rel >= k_rel
            # (one copy per head of the pair so a single strided multiply
            # masks both heads' diagonal blocks)
            mask0 = constp.tile([128, 128], bf16, name="mask0", tag="mask0")
            nc.gpsimd.memset(mask0[:], 1.0)
            nc.gpsimd.affine_select(
                out=mask0[:], in_=mask0[:],
                compare_op=mybir.AluOpType.is_ge, fill=0.0,
                base=0, channel_multiplier=-1, pattern=[[1, 128]],
            )
            mask2 = constp.tile([128, 2, 128], bf16, name="mask2", tag="mask2")
            for _h in range(2):
                nc.gpsimd.tensor_copy(mask2[:, _h, :], mask0[:])

            # V with a ones-column appended per head: [128, sb, head, 64+1]
            vxs = actp.tile([128, _SB, _HPC, 65], bf16, name="vxs", tag="vxs")
            nc.gpsimd.memset(vxs[:, :, :, 64], 1.0)

            qts = [actp.tile([128, _S], bf16, name=f"qt{ft}", tag=f"qt{ft}") for ft in range(_FT)]
            kts = [actp.tile([128, _S], bf16, name=f"kt{ft}", tag=f"kt{ft}") for ft in range(_FT)]
            # ctx as one tile [128, ft, S] so dma-transpose writes all 4
            # feature tiles of a q-block in one instruction
            ctxm = actp.tile([128, _FT, _S], bf16, name="ctxm", tag="ctxm")

            proj_ctr = [0]

            def proj_cp(out_ap, in_ap):
                # alternate psum->sbuf proj copies between DVE and the (early
                # idle) Act engine so the psproj rotation turns around faster
                proj_ctr[0] += 1
                if (opts.get("proj_cp_alt", False)
                        and proj_ctr[0] <= opts.get("proj_cp_n", 12)
                        and proj_ctr[0] % 2):
                    nc.scalar.copy(out_ap, in_ap)
                else:
                    nc.vector.tensor_copy(out_ap, in_ap)

            def proj_v(sb):
                ps = ps_proj.tile([128, 512], f32, tag="psproj", name="psv")
                for dt in range(_DT):
                    nc.tensor.matmul(
                        ps[:], lhsT=xts[:, dt, ts(sb, 128)], rhs=wvs[:, dt, :],
                        start=(dt == 0), stop=(dt == _DT - 1),
                    )
                proj_cp(
                    vxs[:, sb, :, 0:64], ps[:].rearrange("p (h d) -> p h d", h=_HPC)
                )

            def proj_qk1(sc, ft, which):
                wsrc, dst = ((wqs, qts[ft]) if which == 0 else (wks, kts[ft]))
                ps = ps_proj.tile([128, 512], f32, tag="psproj", name="psqk")
                for dt in range(_DT):
                    nc.tensor.matmul(
                        ps[:], lhsT=wsrc[:, dt, ts(ft, 128)],
                        rhs=xts[:, dt, ts(sc, 512)],
                        start=(dt == 0), stop=(dt == _DT - 1),
                    )
                proj_cp(dst[:, ts(sc, 512)], ps[:])

            def chunk_tasks(sc):
                # projection work needed before attention chunk sc runs
                t = [(lambda sb=sb: proj_v(sb)) for sb in range(4 * sc, 4 * sc + 4)]
                t += [(lambda sc=sc, ft=ft, w=w: proj_qk1(sc, ft, w))
                      for ft in range(_FT) for w in range(2)]
                return t

            for t in chunk_tasks(0):
                t()

            def oproj_qb(qb, pools=None, flush=False, last=False):
                # full output row-block qb: two 512-wide psum halves share
                # one ev tile and a single y DMA (halves software-DGE count).
                # The very last block issues per-half DMAs so the first half
                # drains while the second is still in o_proj.
                ev = ev_pool.tile([128, 1024], bf16, tag="ev2", name="ev2")
                for nn2 in range(2):
                    pool, tag = (pools[nn2] if pools else (ps_proj, "psproj"))
                    ps = pool.tile([128, 512], f32, tag=tag, name="pso")
                    for ft in range(_FT):
                        nc.tensor.matmul(
                            ps[:], lhsT=ctxm[:, ft, ts(qb, 128)],
                            rhs=wos[:, ft, ts(nn2, 512)],
                            start=(ft == 0), stop=(ft == _FT - 1),
                        )
                    evsel = ((qb + nn2) if opts.get("ev_qbalt", True) else nn2) % 2
                    cp = (nc.scalar.copy if (opts.get("ev_alt", True) and flush and evsel)
                          else nc.vector.tensor_copy)
                    cp(ev[:, ts(nn2, 512)], ps[:])
                    if last:
                        eng = nc.sync if nn2 else nc.gpsimd
                        eng.dma_start(
                            y[:][ts(qb, 128), ts(nn2, 512)], ev[:, ts(nn2, 512)])
                if not last:
                    # y writes ride the gpsimd software DGE so they never
                    # queue behind a dma-transpose waiting on SP.SEQ
                    eng = (nc.sync if (opts.get("y_alt", True) and flush and qb % 2)
                           else nc.gpsimd)
                    eng.dma_start(y[:][ts(qb, 128), :], ev[:])

            def emit_scores(qc, hp, kbg):
                # scoresT[k, q] = KT_blk.T @ QT_chunk; heads of the pair
                # interleave (row groups 0-63 / 64-127 run concurrently).
                # Non-diagonal kb-groups: h01-major [128, 1024] tiles, one
                # exp per head covering both key blocks. Diagonal groups:
                # kb-major [128, 2, 512] tiles so one 3D-AP exp per key
                # block covers both heads despite the causal column trim.
                soffs = []
                for kbl in range(2):
                    kb = 2 * kbg + kbl
                    off = (kb - 4 * qc) * 128
                    soffs.append(off if off > 0 else 0)
                kb_major = soffs[1] > 0
                scps = [ps_big.tile([128, 2, 512] if kb_major else [128, 1024],
                                    f32, name=f"scp{_i}", tag="scp")
                        for _i in range(2)]
                exs = [exp_pool.tile([128, 2, 512] if kb_major else [128, 1024],
                                     bf16, name=f"ex{_i}", tag="ex")
                       for _i in range(2)]
                # matmul order matches act coverage: h01-major tiles pair
                # (h01, both kbls), kb-major tiles pair (kbl, both h01s) --
                # either way the first act starts after two matmuls
                order = ([(h01, kbl) for kbl in range(2) for h01 in range(2)]
                         if kb_major else
                         [(h01, kbl) for h01 in range(2) for kbl in range(2)])
                for h01, kbl in order:
                    pb = 64 * h01
                    kb = 2 * kbg + kbl
                    soff = soffs[kbl]
                    dst = (scps[kbl][:, h01, soff:512] if kb_major
                           else scps[h01][:, kbl * 512 + soff:(kbl + 1) * 512])
                    nc.tensor.matmul(
                        dst,
                        lhsT=kts[hp][pb:pb + 64, ts(kb, 128)],
                        rhs=qts[hp][pb:pb + 64, qc * 512 + soff:(qc + 1) * 512],
                        start=True, stop=True,
                    )
                if kb_major:
                    for kbl in range(2):
                        soff = soffs[kbl]
                        nc.scalar.activation(
                            exs[kbl][:, :, soff:512], scps[kbl][:, :, soff:512], Exp)
                else:
                    for h01 in range(2):
                        nc.scalar.activation(
                            exs[h01][:, soffs[0]:1024],
                            scps[h01][:, soffs[0]:1024], Exp)
                return (kb_major, exs)

            def emit_av(qc, hp, kbg, ex_info, avs):
                # diagonal masks first, then the AV matmuls in transposed
                # orientation: avs[h01][q-part, qb, 65] accumulates
                # lhsT=expT[k, q-block] x rhs=V[k, 65] over key blocks.
                kb_major, exs = ex_info
                maskeng = nc.gpsimd if opts.get("mask_gpsimd") else nc.vector
                for kbl in range(2):
                    kb = 2 * kbg + kbl
                    off = (kb - 4 * qc) * 128
                    if off < 0:
                        continue
                    if kb_major:
                        if opts.get("mask_split"):
                            for h01 in range(2):
                                maskeng.tensor_mul(
                                    exs[kbl][:, h01, off:off + 128],
                                    exs[kbl][:, h01, off:off + 128],
                                    mask0[:],
                                )
                        else:
                            maskeng.tensor_mul(
                                exs[kbl][:, :, off:off + 128],
                                exs[kbl][:, :, off:off + 128],
                                mask2[:],
                            )
                    else:
                        for h01 in range(2):
                            maskeng.tensor_mul(
                                exs[h01][:, kbl * 512 + off:kbl * 512 + off + 128],
                                exs[h01][:, kbl * 512 + off:kbl * 512 + off + 128],
                                mask0[:],
                            )
                # psum "zero regions" are bank-granular: exactly ONE
                # start (first matmul into the tile: it marks the whole
                # bank pending-zero, so each chain's first write zeroes
                # its own bytes) and ONE stop (very last matmul).
                ao = opts.get("av_order", 0)
                if ao == 0:
                    order = [(h01, kbl, qb) for h01 in range(2)
                             for kbl in range(2) for qb in range(4)]
                elif ao == 1:
                    order = [(h01, kbl, qb) for kbl in range(2)
                             for h01 in range(2) for qb in range(4)]
                else:
                    order = [(h01, kbl, qb) for kbl in range(2)
                             for qb in range(4) for h01 in range(2)]
                for h01, kbl, qb in order:
                    kb = 2 * kbg + kbl
                    off = (kb - 4 * qc) * 128
                    h = 2 * hp + h01
                    if qb * 128 < off:
                        continue  # block fully above the diagonal
                    lhsT = (exs[kbl][:, h01, qb * 128:(qb + 1) * 128]
                            if kb_major else
                            exs[h01][:, kbl * 512 + qb * 128:kbl * 512 + (qb + 1) * 128])
                    nc.tensor.matmul(
                        avs[h01][:, qb, :],
                        lhsT=lhsT,
                        rhs=vxs[:, kb, h, :],
                        start=(kb == 0 and qb == 0),
                        stop=(kb == 4 * qc + 3 and qb == 3),
                    )

            def emit_norm_qbs(qc, hp, avs, stage, qbs):
                # norm a pair of q-blocks as soon as their AV chains stop
                # (used on the very last pass so transposes/o_proj overlap
                # the exp drain instead of serializing after it)
                for h01 in range(2):
                    rc = nrm_pool.tile([128, 2], f32, tag="rc2")
                    nc.vector.reciprocal(rc[:], avs[h01][:, qbs[0]:qbs[0] + 2, 64])
                    for i, qb in enumerate(qbs):
                        nc.vector.tensor_scalar_mul(
                            stage[:, qb, hp * 128 + h01 * 64:hp * 128 + h01 * 64 + 64],
                            avs[h01][:, qb, 0:64],
                            rc[:, i:i + 1],
                        )

            def emit_transpose_qb(qc, stage, qb):
                nc.sync.dma_start_transpose(
                    ctxm[:, :, qc * 512 + qb * 128:qc * 512 + (qb + 1) * 128],
                    stage[:, qb, :],
                )

            def emit_norm(qc, hp, avs, stage):
                # ctx[q, f] = avU[q, f] / sumexp[q]: per-partition scalar
                # multiply by the reciprocal of the fused ones-column.
                from concourse.bass import broadcast_tensor_aps
                for h01 in range(2):
                    rc = nrm_pool.tile([128, 4], f32, tag="rc")
                    nc.vector.reciprocal(rc[:], avs[h01][:, :, 64])
                    eng = (nc.gpsimd if (opts.get("norm_split") and h01)
                           else nc.vector)
                    if opts.get("norm_bcast", False):
                        # one mul for all 4 q-blocks: rc broadcast along the
                        # feature dim with a stride-0 AP
                        out_ap = stage[:, :, hp * 128 + h01 * 64:hp * 128 + h01 * 64 + 64]
                        in0 = avs[h01][:, :, 0:64]
                        in1 = rc[:].rearrange("p (q o) -> p q o", o=1)
                        in0b, in1b = broadcast_tensor_aps(in0, in1)
                        eng.tensor_mul(out_ap, in0b, in1b)
                    else:
                        for qb in range(4):
                            eng.tensor_scalar_mul(
                                stage[:, qb, hp * 128 + h01 * 64:hp * 128 + h01 * 64 + 64],
                                avs[h01][:, qb, 0:64],
                                rc[:, qb:qb + 1],
                            )

            def emit_transpose(qc, stage):
                # [q, f] -> [f, q] on the DMA crossbar: zero PE/DVE cost.
                # Last chunk alternates SP/Act issue so the four transposes
                # don't serialize on one sequencer right before the flush.
                for qb in range(4):
                    eng = (nc.scalar if (opts.get("tp_alt") and qc == _QC - 1 and qb % 2)
                           else nc.sync)
                    eng.dma_start_transpose(
                        ctxm[:, :, qc * 512 + qb * 128:qc * 512 + (qb + 1) * 128],
                        stage[:, qb, :],
                    )

            pending = []
            soft = []  # o_proj work: deferred to the end flush (measured fastest)

            def finish_unit(qc, hp, kbg, ex_info):
                key = (qc, hp)
                avs = avs_by_hp.get(key)
                if avs is None:
                    avs = [ps_av.tile([128, 4, 65], f32, name=f"av{_i}", tag="av")
                           for _i in range(2)]
                    avs_by_hp[key] = avs
                emit_av(qc, hp, kbg, ex_info, avs)
                last_pass = (qc == _QC - 1 and hp == _FT - 1
                             and opts.get("early_tail", False))
                stage = stage_by_qc.get(qc)
                if stage is None:
                    stage = stg_pool.tile([128, 4, 512], bf16, tag="stage")
                    stage_by_qc[qc] = stage
                if last_pass and kbg == 2 * qc:
                    emit_norm_qbs(qc, hp, avs, stage, (0, 1))
                    emit_transpose_qb(qc, stage, 0)
                    emit_transpose_qb(qc, stage, 1)
                if kbg == 2 * qc + 1:
                    avs_by_hp.pop(key)
                    if last_pass:
                        emit_norm_qbs(qc, hp, avs, stage, (2, 3))
                        emit_transpose_qb(qc, stage, 2)
                        emit_transpose_qb(qc, stage, 3)
                        stage_by_qc.pop(qc)
                        flush_final(list(range(4 * qc, 4 * qc + 4)))
                    else:
                        emit_norm(qc, hp, avs, stage)
                        if hp == _FT - 1:
                            emit_transpose(qc, stage_by_qc.pop(qc))
                            ot = list(range(4 * qc, 4 * qc + 4))
                            if qc + 1 < _QC:
                                soft.extend(ot)
                            else:
                                flush_final(ot)

            def flush_final(otasks):
                # final flush: rotate the finished chunk's o_proj
                # groups across all freed psum pools
                rot = [(ps_proj, "psproj"), (ps_big, "scp"), (ps_av, "av")]
                for qb in soft:
                    oproj_qb(qb, flush=True)
                nlast = opts.get("tail_n", 4)
                for i, qb in enumerate(otasks):
                    pools = (rot[(2 * i) % 3], rot[(2 * i + 1) % 3])
                    oproj_qb(qb, pools, flush=True,
                             last=(opts.get("tail_split", True)
                                   and i >= len(otasks) - nlast))

            # software-pipeline the emission across hp AND qc
            # boundaries: scores of the next kb-group enter the PE
            # stream before AV of the previous, so PE never stalls
            # on exp latency.
            inflight = []
            avs_by_hp = {}
            stage_by_qc = {}
            kb_ctr = 0
            for qc in range(_QC):
                pending.extend(chunk_tasks(qc + 1) if qc + 1 < _QC else [])
                kbs_left = _FT * (2 * qc + 2)
                for hp in range(_FT):
                    for kbg in range(2 * qc + 2):
                        # interleave next chunk's projection work
                        npop = -(-len(pending) // kbs_left) if pending else 0
                        kbs_left -= 1
                        for _ in range(npop):
                            pending.pop(0)()
                        drip_ok = (qc == _QC - 1) if opts.get("drip_qc3") else True
                        if (soft and not pending and drip_ok
                                and kb_ctr % opts.get("soft_rate", 8) == 0):
                            oproj_qb(soft.pop(0))
                        kb_ctr += 1
                        inflight.append((qc, hp, kbg, emit_scores(qc, hp, kbg)))
                        if len(inflight) >= opts.get("depth", 9):
                            q0, h0, k0, e0 = inflight.pop(0)
                            finish_unit(q0, h0, k0, e0)
            for q0, h0, k0, e0 in inflight:
                finish_unit(q0, h0, k0, e0)

            if opts.get("dbg"):
                ydbg = nc.dram_tensor("ctx_dbg", [128, _FT, _S], bf16,
                                      kind="ExternalOutput")
                nc.sync.dma_start(ydbg[:], ctxm[:])
                qdbg = nc.dram_tensor("q_dbg", [128, _FT, _S], bf16,
                                      kind="ExternalOutput")
                for ft in range(_FT):
                    nc.sync.dma_start(qdbg[:][:, ft, :], qts[ft][:])
                kdbg = nc.dram_tensor("k_dbg", [128, _FT, _S], bf16,
                                      kind="ExternalOutput")
                for ft in range(_FT):
                    nc.sync.dma_start(kdbg[:][:, ft, :], kts[ft][:])
                vdbg = nc.dram_tensor("v_dbg", [128, _SB, _HPC, 65], bf16,
                                      kind="ExternalOutput")
                nc.sync.dma_start(vdbg[:], vxs[:])

    nc.compile()
    return nc


def _get_nc(opts=None):
    key = tuple(sorted((opts or {}).items()))
    if key not in _cache:
        _cache[key] = _build_nc(opts)
    return _cache[key]


def _shard(x, Wq, Wk, Wv, Wo):
    in_maps = []
    for c in range(_NCORES):
        b, hh = divmod(c, 2)
        cols = slice(512 * hh, 512 * hh + 512)
        in_maps.append({
            "xt": np.ascontiguousarray(x[b].T).astype(_BF16),
            "wq": (Wq[:, cols] * np.float32(0.125)).astype(_BF16),
            "wk": np.ascontiguousarray(Wk[:, cols]).astype(_BF16),
            "wv": np.ascontiguousarray(Wv[:, cols]).astype(_BF16),
            "wo": np.ascontiguousarray(Wo[cols, :]).astype(_BF16),
        })
    return in_maps


def _run(inputs, trace=False):
    from concourse import bass_utils

    x = np.asarray(inputs["x"], dtype=np.float32)
    Wq = np.asarray(inputs["Wq"], dtype=np.float32)
    Wk = np.asarray(inputs["Wk"], dtype=np.float32)
    Wv = np.asarray(inputs["Wv"], dtype=np.float32)
    Wo = np.asarray(inputs["Wo"], dtype=np.float32)
    # attention_mask is all-ones by problem spec (fill=ones) -> no-op.

    nc = _get_nc()
    res = bass_utils.run_bass_kernel_spmd(
        nc, _shard(x, Wq, Wk, Wv, Wo), core_ids=list(range(_NCORES)), trace=trace
    )
    ys = [np.asarray(r["y"]).astype(np.float32) for r in res.results]
    out = np.stack([ys[2 * b] + ys[2 * b + 1] for b in range(_B)])
    return out, res


def kernel(**inputs):
    return _run(inputs)[0]


# revision 73
# speedup vs baseline: 1.0387x; 1.0359x over previous
"""Multi-head causal self-attention on 8 Trainium2 NeuronCores.

Problem: x[4,2048,1024] fp32, Wq/Wk/Wv/Wo[1024,1024], H=16 heads, head_dim=64,
causal mask, attention_mask all-ones (per spec fill=ones -> no-op).

Sharding (hybrid data/tensor parallel):
  core c -> batch b = c//2, head-half hh = c%2 (8 heads = 512 features).
  Each core: Q/K/V projections with column-sliced W (Megatron column
  parallel), attention for its 8 heads, o_proj with row-sliced Wo
  (row parallel) producing a partial [2048,1024] output. The host sums
  the two partials per batch (the "all-reduce") and stacks batches.

Device kernel (per core), all matmuls bf16 in / fp32 PSUM accumulate:
  QT/KT produced in transposed [feat, seq] layout directly (x is fed
  pre-transposed from the host), scores are computed transposed
  (scoresT[k,q] = KT_blk.T @ QT) so softmax needs no on-chip transpose:
  exp runs on ScalarE straight out of PSUM.  AV runs in the transposed
  orientation: out[q, hd] accumulates lhsT=expT[k, q-block] x rhs=V[k, hd]
  per 128-wide q-block, so each accumulation step is charged only 65
  output columns (64 v-dims + a fused ones-column giving sumexp) instead
  of 512 -- half the PE work of the untransposed form.  Normalization is
  then a per-partition scalar multiply (DVE tensor_scalar) into a bf16
  staging tile, and the [q, feat] -> [feat, q] layout flip that o_proj
  needs is done by the DMA crossbar (dma_start_transpose), which costs
  zero PE/DVE time.  Causal handling: block-skip fully-masked key blocks,
  one 128x128 triangular mask multiply on diagonal blocks.
"""

import numpy as np
import ml_dtypes

_BF16 = ml_dtypes.bfloat16
_B, _S, _D = 4, 2048, 1024
_NCORES = 8
_HPC = 8   # heads per core
_FT = 4    # 128-wide feature tiles per core (= head pairs)
_DT = 8    # 128-wide tiles of D
_SB = 16   # 128-wide seq blocks
_QC = 4    # 512-wide seq chunks

_cache = {}


def _build_nc(opts=None):
    opts = opts or {}
    import concourse.bacc as bacc
    import concourse.mybir as mybir
    import concourse.tile as tile
    from concourse.bass import ts

    f32 = mybir.dt.float32
    bf16 = mybir.dt.bfloat16
    Exp = mybir.ActivationFunctionType.Exp

    nc = bacc.Bacc("TRN2", target_bir_lowering=False, debug=False)

    xt = nc.dram_tensor("xt", [_D, _S], bf16, kind="ExternalInput")   # x[b].T
    wq = nc.dram_tensor("wq", [_D, 512], bf16, kind="ExternalInput")  # pre-scaled 1/8
    wk = nc.dram_tensor("wk", [_D, 512], bf16, kind="ExternalInput")
    wv = nc.dram_tensor("wv", [_D, 512], bf16, kind="ExternalInput")
    wo = nc.dram_tensor("wo", [512, _D], bf16, kind="ExternalInput")
    # y partials in bf16: host accumulates in fp32; halves output DMA time
    y = nc.dram_tensor("y", [_S, _D], bf16, kind="ExternalOutput")

    with tile.TileContext(nc) as tc:
        with (
            tc.tile_pool(name="const", bufs=1) as constp,
            tc.tile_pool(name="win", bufs=1) as wp,
            tc.tile_pool(name="acts", bufs=1) as actp,
            tc.tile_pool(name="ex", bufs=opts.get("ex_bufs", 18)) as exp_pool,
            tc.tile_pool(name="ev", bufs=opts.get("ev_bufs", 6)) as ev_pool,
            tc.tile_pool(name="stg", bufs=opts.get("stg_bufs", 2)) as stg_pool,
            tc.tile_pool(name="nrm", bufs=opts.get("nrm_bufs", 4)) as nrm_pool,
            tc.tile_pool(name="ps_proj", bufs=opts.get("proj_bufs", 2), space="PSUM") as ps_proj,
            tc.tile_pool(name="ps_big", bufs=2, space="PSUM") as ps_big,
            tc.tile_pool(name="ps_av", bufs=opts.get("av_bufs", 2), space="PSUM") as ps_av,
        ):
            # ---- input loads, split and ordered by first use so the first
            # matmuls start after ~1/8 of the bytes land ---------------------
            xts = wp.tile([128, _DT, _S], bf16, name="xts", tag="xts")
            wqs = wp.tile([128, _DT, 512], bf16, name="wqs", tag="wqs")
            wks = wp.tile([128, _DT, 512], bf16, name="wks", tag="wks")
            wvs = wp.tile([128, _DT, 512], bf16, name="wvs", tag="wvs")
            wos = wp.tile([128, _FT, _D], bf16, name="wos", tag="wos")
            xt_r = xt[:].rearrange("(dt p) s -> p dt s", p=128)
            wq_r = wq[:].rearrange("(dt p) n -> p dt n", p=128)
            wk_r = wk[:].rearrange("(dt p) n -> p dt n", p=128)
            wv_r = wv[:].rearrange("(dt p) n -> p dt n", p=128)
            # three DGE paths in parallel: SP streams wv (+late xt chunks),
            # Act streams xt chunk 0, gpsimd's software DGE streams wq/wk --
            # the 650ns/issue sequencer serialization stops gating startup
            dm = opts.get("dma_mode", 0)
            if dm == 3:
                # wq/wk dh0 woven between the wv/xt pieces so the first
                # q/k projection chains start ~3us earlier
                for dts in (slice(0, 1), slice(1, 2)):
                    nc.sync.dma_start(wvs[:, dts], wv_r[:, dts])
                    nc.sync.dma_start(xts[:, dts, 0:512], xt_r[:, dts, 0:512])
                nc.sync.dma_start(wqs[:, 0:4], wq_r[:, 0:4])
                nc.sync.dma_start(wvs[:, 2:4], wv_r[:, 2:4])
                nc.sync.dma_start(xts[:, 2:4, 0:512], xt_r[:, 2:4, 0:512])
                nc.sync.dma_start(wks[:, 0:4], wk_r[:, 0:4])
                nc.sync.dma_start(wvs[:, 4:8], wv_r[:, 4:8])
                nc.sync.dma_start(xts[:, 4:8, 0:512], xt_r[:, 4:8, 0:512])
                nc.sync.dma_start(wqs[:, 4:8], wq_r[:, 4:8])
                nc.sync.dma_start(wks[:, 4:8], wk_r[:, 4:8])
            else:
                xt_eng = nc.scalar if dm == 1 else nc.sync
                qk_eng = nc.scalar if dm == 4 else (nc.gpsimd if dm == 2 else nc.sync)
                for dts in (slice(0, 1), slice(1, 2), slice(2, 4), slice(4, 8)):
                    nc.sync.dma_start(wvs[:, dts], wv_r[:, dts])
                    xt_eng.dma_start(xts[:, dts, 0:512], xt_r[:, dts, 0:512])
                for dh in range(2):
                    dts = slice(dh * 4, dh * 4 + 4)
                    qk_eng.dma_start(wqs[:, dts], wq_r[:, dts])
                    qk_eng.dma_start(wks[:, dts], wk_r[:, dts])
            for sc in range(1, _QC):
                nc.sync.dma_start(
                    xts[:, :, ts(sc, 512)], xt_r[:, :, ts(sc, 512)])
            nc.sync.dma_start(wos[:], wo[:].rearrange("(ft p) n -> p ft n", p=128))

            # causal mask for diagonal 128x128 blocks: keep iff q_rel >= k_rel
            # (one copy per head of the pair so a single strided multiply
            # masks both heads' diagonal blocks)
            mask0 = constp.tile([128, 128], bf16, name="mask0", tag="mask0")
            nc.gpsimd.memset(mask0[:], 1.0)
            nc.gpsimd.affine_select(
                out=mask0[:], in_=mask0[:],
                compare_op=mybir.AluOpType.is_ge, fill=0.0,
                base=0, channel_multiplier=-1, pattern=[[1, 128]],
            )
            mask2 = constp.tile([128, 2, 128], bf16, name="mask2", tag="mask2")
            for _h in range(2):
                nc.gpsimd.tensor_copy(mask2[:, _h, :], mask0[:])

            # V with a ones-column appended per head: [128, sb, head, 64+1]
            vxs = actp.tile([128, _SB, _HPC, 65], bf16, name="vxs", tag="vxs")
            nc.gpsimd.memset(vxs[:, :, :, 64], 1.0)

            qts = [actp.tile([128, _S], bf16, name=f"qt{ft}", tag=f"qt{ft}") for ft in range(_FT)]
            kts = [actp.tile([128, _S], bf16, name=f"kt{ft}", tag=f"kt{ft}") for ft in range(_FT)]
            # ctx as one tile [128, ft, S] so dma-transpose writes all 4
            # feature tiles of a q-block in one instruction
            ctxm = actp.tile([128, _FT, _S], bf16, name="ctxm", tag="ctxm")

            proj_ctr = [0]

            def proj_cp(out_ap, in_ap):
                # alternate psum->sbuf proj copies between DVE and the (early
                # idle) Act engine so the psproj rotation turns around faster
                proj_ctr[0] += 1
                if (opts.get("proj_cp_alt", False)
                        and proj_ctr[0] <= opts.get("proj_cp_n", 12)
                        and proj_ctr[0] % 2):
                    nc.scalar.copy(out_ap, in_ap)
                else:
                    nc.vector.tensor_copy(out_ap, in_ap)

            def proj_v(sb):
                ps = ps_proj.tile([128, 512], f32, tag="psproj", name="psv")
                for dt in range(_DT):
                    nc.tensor.matmul(
                        ps[:], lhsT=xts[:, dt, ts(sb, 128)], rhs=wvs[:, dt, :],
                        start=(dt == 0), stop=(dt == _DT - 1),
                    )
                proj_cp(
                    vxs[:, sb, :, 0:64], ps[:].rearrange("p (h d) -> p h d", h=_HPC)
                )

            def proj_qk1(sc, ft, which):
                wsrc, dst = ((wqs, qts[ft]) if which == 0 else (wks, kts[ft]))
                ps = ps_proj.tile([128, 512], f32, tag="psproj", name="psqk")
                for dt in range(_DT):
                    nc.tensor.matmul(
                        ps[:], lhsT=wsrc[:, dt, ts(ft, 128)],
                        rhs=xts[:, dt, ts(sc, 512)],
                        start=(dt == 0), stop=(dt == _DT - 1),
                    )
                proj_cp(dst[:, ts(sc, 512)], ps[:])

            def chunk_tasks(sc):
                # projection work needed before attention chunk sc runs
                t = [(lambda sb=sb: proj_v(sb)) for sb in range(4 * sc, 4 * sc + 4)]
                t += [(lambda sc=sc, ft=ft, w=w: proj_qk1(sc, ft, w))
                      for ft in range(_FT) for w in range(2)]
                return t

            for t in chunk_tasks(0):
                t()

            def oproj_qb(qb, pools=None, flush=False, last=False):
                # full output row-block qb: two 512-wide psum halves share
                # one ev tile and a single y DMA (halves software-DGE count).
                # The very last block issues per-half DMAs so the first half
                # drains while the second is still in o_proj.
                ev = ev_pool.tile([128, 1024], bf16, tag="ev2", name="ev2")
                for nn2 in range(2):
                    pool, tag = (pools[nn2] if pools else (ps_proj, "psproj"))
                    ps = pool.tile([128, 512], f32, tag=tag, name="pso")
                    for ft in range(_FT):
                        nc.tensor.matmul(
                            ps[:], lhsT=ctxm[:, ft, ts(qb, 128)],
                            rhs=wos[:, ft, ts(nn2, 512)],
                            start=(ft == 0), stop=(ft == _FT - 1),
                        )
                    evsel = ((qb + nn2) if opts.get("ev_qbalt", True) else nn2) % 2
                    cp = (nc.scalar.copy if (opts.get("ev_alt", True) and flush and evsel)
                          else nc.vector.tensor_copy)
                    cp(ev[:, ts(nn2, 512)], ps[:])
                    if last:
                        eng = nc.sync if nn2 else nc.gpsimd
                        eng.dma_start(
                            y[:][ts(qb, 128), ts(nn2, 512)], ev[:, ts(nn2, 512)])
                if not last:
                    # y writes ride the gpsimd software DGE so they never
                    # queue behind a dma-transpose waiting on SP.SEQ
                    eng = (nc.sync if (opts.get("y_alt", True) and flush and qb % 2)
                           else nc.gpsimd)
                    eng.dma_start(y[:][ts(qb, 128), :], ev[:])

            def emit_scores(qc, hp, kbg):
                # scoresT[k, q] = KT_blk.T @ QT_chunk; heads of the pair
                # interleave (row groups 0-63 / 64-127 run concurrently).
                # Non-diagonal kb-groups: h01-major [128, 1024] tiles, one
                # exp per head covering both key blocks. Diagonal groups:
                # kb-major [128, 2, 512] tiles so one 3D-AP exp per key
                # block covers both heads despite the causal column trim.
                soffs = []
                for kbl in range(2):
                    kb = 2 * kbg + kbl
                    off = (kb - 4 * qc) * 128
                    soffs.append(off if off > 0 else 0)
                kb_major = soffs[1] > 0
                scps = [ps_big.tile([128, 2, 512] if kb_major else [128, 1024],
                                    f32, name=f"scp{_i}", tag="scp")
                        for _i in range(2)]
                exs = [exp_pool.tile([128, 2, 512] if kb_major else [128, 1024],
                                     bf16, name=f"ex{_i}", tag="ex")
                       for _i in range(2)]
                # matmul order matches act coverage: h01-major tiles pair
                # (h01, both kbls), kb-major tiles pair (kbl, both h01s) --
                # either way the first act starts after two matmuls
                order = ([(h01, kbl) for kbl in range(2) for h01 in range(2)]
                         if kb_major else
                         [(h01, kbl) for h01 in range(2) for kbl in range(2)])
                for h01, kbl in order:
                    pb = 64 * h01
                    kb = 2 * kbg + kbl
                    soff = soffs[kbl]
                    dst = (scps[kbl][:, h01, soff:512] if kb_major
                           else scps[h01][:, kbl * 512 + soff:(kbl + 1) * 512])
                    nc.tensor.matmul(
                        dst,
                        lhsT=kts[hp][pb:pb + 64, ts(kb, 128)],
                        rhs=qts[hp][pb:pb + 64, qc * 512 + soff:(qc + 1) * 512],
                        start=True, stop=True,
                    )
                if kb_major:
                    for kbl in range(2):
                        soff = soffs[kbl]
                        nc.scalar.activation(
                            exs[kbl][:, :, soff:512], scps[kbl][:, :, soff:512], Exp)
                else:
                    for h01 in range(2):
                        nc.scalar.activation(
                            exs[h01][:, soffs[0]:1024],
                            scps[h01][:, soffs[0]:1024], Exp)
                return (kb_major, exs)

            def emit_av(qc, hp, kbg, ex_info, avs):
                # diagonal masks first, then the AV matmuls in transposed
                # orientation: avs[h01][q-part, qb, 65] accumulates
                # lhsT=expT[k, q-block] x rhs=V[k, 65] over key blocks.
                kb_major, exs = ex_info
                maskeng = nc.gpsimd if opts.get("mask_gpsimd") else nc.vector
                for kbl in range(2):
                    kb = 2 * kbg + kbl
                    off = (kb - 4 * qc) * 128
                    if off < 0:
                        continue
                    if kb_major:
                        if opts.get("mask_split"):
                            for h01 in range(2):
                                maskeng.tensor_mul(
                                    exs[kbl][:, h01, off:off + 128],
                                    exs[kbl][:, h01, off:off + 128],
                                    mask0[:],
                                )
                        else:
                            maskeng.tensor_mul(
                                exs[kbl][:, :, off:off + 128],
                                exs[kbl][:, :, off:off + 128],
                                mask2[:],
                            )
                    else:
                        for h01 in range(2):
                            maskeng.tensor_mul(
                                exs[h01][:, kbl * 512 + off:kbl * 512 + off + 128],
                                exs[h01][:, kbl * 512 + off:kbl * 512 + off + 128],
                                mask0[:],
                            )
                # psum "zero regions" are bank-granular: exactly ONE
                # start (first matmul into the tile: it marks the whole
                # bank pending-zero, so each chain's first write zeroes
                # its own bytes) and ONE stop (very last matmul).
                ao = opts.get("av_order", 0)
                if ao == 0:
                    order = [(h01, kbl, qb) for h01 in range(2)
                             for kbl in range(2) for qb in range(4)]
                elif ao == 1:
                    order = [(h01, kbl, qb) for kbl in range(2)
                             for h01 in range(2) for qb in range(4)]
                else:
                    order = [(h01, kbl, qb) for kbl in range(2)
                             for qb in range(4) for h01 in range(2)]
                # no start/stop at all: the tiles are memset-zeroed at
                # allocation and every matmul is a pure accumulate.  A
                # start flag here would mark the tile's WHOLE psum bank
                # pending-zero -- a cross-region hazard the tile
                # framework's subtile dependency tracking cannot see when
                # passes interleave.
                for h01, kbl, qb in order:
                    kb = 2 * kbg + kbl
                    off = (kb - 4 * qc) * 128
                    h = 2 * hp + h01
                    if qb * 128 < off:
                        continue  # block fully above the diagonal
                    lhsT = (exs[kbl][:, h01, qb * 128:(qb + 1) * 128]
                            if kb_major else
                            exs[h01][:, kbl * 512 + qb * 128:kbl * 512 + (qb + 1) * 128])
                    if opts.get("av_memset", True):
                        nc.tensor.matmul(
                            avs[h01][:, qb, :], lhsT=lhsT, rhs=vxs[:, kb, h, :],
                            start=False, stop=False, skip_group_check=True,
                        )
                    else:
                        nc.tensor.matmul(
                            avs[h01][:, qb, :], lhsT=lhsT, rhs=vxs[:, kb, h, :],
                            start=(kb == 0 and qb == 0),
                            stop=(kb == 4 * qc + 3 and qb == 3),
                        )

            def emit_norm_qbs(qc, hp, avs, stage, qbs):
                # norm a pair of q-blocks as soon as their AV chains stop
                # (used on the very last pass so transposes/o_proj overlap
                # the exp drain instead of serializing after it)
                for h01 in range(2):
                    rc = nrm_pool.tile([128, 2], f32, tag="rc2")
                    nc.vector.reciprocal(rc[:], avs[h01][:, qbs[0]:qbs[0] + 2, 64])
                    for i, qb in enumerate(qbs):
                        nc.vector.tensor_scalar_mul(
                            stage[:, qb, hp * 128 + h01 * 64:hp * 128 + h01 * 64 + 64],
                            avs[h01][:, qb, 0:64],
                            rc[:, i:i + 1],
                        )

            def emit_transpose_qb(qc, stage, qb):
                nc.sync.dma_start_transpose(
                    ctxm[:, :, qc * 512 + qb * 128:qc * 512 + (qb + 1) * 128],
                    stage[:, qb, :],
                )

            def emit_norm(qc, hp, avs, stage):
                # ctx[q, f] = avU[q, f] / sumexp[q]: per-partition scalar
                # multiply by the reciprocal of the fused ones-column.
                from concourse.bass import broadcast_tensor_aps
                for h01 in range(2):
                    rc = nrm_pool.tile([128, 4], f32, tag="rc")
                    nc.vector.reciprocal(rc[:], avs[h01][:, :, 64])
                    eng = (nc.gpsimd if (opts.get("norm_split") and h01)
                           else nc.vector)
                    if opts.get("norm_bcast", False):
                        # one mul for all 4 q-blocks: rc broadcast along the
                        # feature dim with a stride-0 AP
                        out_ap = stage[:, :, hp * 128 + h01 * 64:hp * 128 + h01 * 64 + 64]
                        in0 = avs[h01][:, :, 0:64]
                        in1 = rc[:].rearrange("p (q o) -> p q o", o=1)
                        in0b, in1b = broadcast_tensor_aps(in0, in1)
                        eng.tensor_mul(out_ap, in0b, in1b)
                    else:
                        for qb in range(4):
                            eng.tensor_scalar_mul(
                                stage[:, qb, hp * 128 + h01 * 64:hp * 128 + h01 * 64 + 64],
                                avs[h01][:, qb, 0:64],
                                rc[:, qb:qb + 1],
                            )

            def emit_transpose(qc, stage):
                # [q, f] -> [f, q] on the DMA crossbar: zero PE/DVE cost.
                # Last chunk alternates SP/Act issue so the four transposes
                # don't serialize on one sequencer right before the flush.
                for qb in range(4):
                    eng = (nc.scalar if (opts.get("tp_alt") and qc == _QC - 1 and qb % 2)
                           else nc.sync)
                    eng.dma_start_transpose(
                        ctxm[:, :, qc * 512 + qb * 128:qc * 512 + (qb + 1) * 128],
                        stage[:, qb, :],
                    )

            pending = []
            soft = []  # o_proj work: deferred to the end flush (measured fastest)

            final_pass = [None]  # set once the pass schedule is chosen

            def finish_unit(qc, hp, kbg, ex_info):
                key = (qc, hp)
                avs = avs_by_hp.get(key)
                if avs is None:
                    avs = [ps_av.tile([128, 4, 65], f32, name=f"av{_i}", tag="av")
                           for _i in range(2)]
                    if opts.get("av_memset", True):
                        for _a in avs:
                            nc.vector.memset(_a[:], 0.0)
                    avs_by_hp[key] = avs
                emit_av(qc, hp, kbg, ex_info, avs)
                # "final" = the last pass of the WHOLE schedule (the flush
                # must run there, whatever chunk order is in effect)
                is_final = (qc, hp) == final_pass[0]
                last_pass = is_final and opts.get("early_tail", False)
                stage = stage_by_qc.get(qc)
                if stage is None:
                    stage = stg_pool.tile([128, 4, 512], bf16, tag="stage")
                    stage_by_qc[qc] = stage
                if last_pass and kbg == 2 * qc:
                    emit_norm_qbs(qc, hp, avs, stage, (0, 1))
                    emit_transpose_qb(qc, stage, 0)
                    emit_transpose_qb(qc, stage, 1)
                if kbg == 2 * qc + 1:
                    avs_by_hp.pop(key)
                    if last_pass:
                        emit_norm_qbs(qc, hp, avs, stage, (2, 3))
                        emit_transpose_qb(qc, stage, 2)
                        emit_transpose_qb(qc, stage, 3)
                        stage_by_qc.pop(qc)
                        flush_final(list(range(4 * qc, 4 * qc + 4)))
                    else:
                        emit_norm(qc, hp, avs, stage)
                        if hp == _FT - 1:
                            emit_transpose(qc, stage_by_qc.pop(qc))
                            ot = list(range(4 * qc, 4 * qc + 4))
                            if is_final:
                                flush_final(ot)
                            else:
                                soft.extend(ot)

            def flush_final(otasks):
                # final flush: rotate the finished chunk's o_proj
                # groups across all freed psum pools
                rot = [(ps_proj, "psproj"), (ps_big, "scp"), (ps_av, "av")]
                for qb in soft:
                    oproj_qb(qb, flush=True)
                nlast = opts.get("tail_n", 4)
                for i, qb in enumerate(otasks):
                    pools = (rot[(2 * i) % 3], rot[(2 * i + 1) % 3])
                    oproj_qb(qb, pools, flush=True,
                             last=(opts.get("tail_split", True)
                                   and i >= len(otasks) - nlast))

            # software-pipeline the emission across hp AND qc
            # boundaries: scores of the next kb-group enter the PE
            # stream before AV of the previous, so PE never stalls
            # on exp latency.
            inflight = []
            avs_by_hp = {}
            stage_by_qc = {}
            kb_ctr = 0

            # Pass schedule. Sequential chunk order concentrates all the
            # exp-heavy (Act-bound) chunk-3 passes at the end with no
            # projection filler left for PE; the interleaved order pairs
            # each chunk-3 pass with lighter passes and forced projection
            # bursts so PE stays the pacing engine throughout.
            om = opts.get("order", 6)
            if om == 1:
                passes = [(0, 0), (0, 1), (0, 2), (0, 3),
                          (1, 0), (1, 1), (3, 0), (1, 2), (1, 3), (3, 1),
                          (2, 0), (2, 1), (3, 2), (2, 2), (2, 3), (3, 3)]
            elif om == 2:
                passes = [(0, 0), (0, 1), (0, 2), (0, 3),
                          (1, 0), (1, 1), (1, 2), (1, 3),
                          (2, 0), (2, 1), (2, 2), (3, 0), (2, 3),
                          (3, 1), (3, 2), (3, 3)]
            elif om == 3:
                passes = ([(0, h) for h in range(4)] + [(1, h) for h in range(4)]
                          + [(3, h) for h in range(4)] + [(2, h) for h in range(4)])
            elif om == 4:
                passes = [(0, 0), (0, 1), (0, 2), (0, 3),
                          (1, 0), (1, 1), (3, 0), (1, 2), (3, 1), (1, 3),
                          (3, 2), (2, 0), (2, 1), (2, 2), (2, 3), (3, 3)]
            elif om == 5:
                passes = [(0, 0), (0, 1), (0, 2), (0, 3),
                          (1, 0), (1, 1), (1, 2), (3, 0), (1, 3),
                          (2, 0), (3, 1), (2, 1), (3, 2), (2, 2), (2, 3), (3, 3)]
            elif om == 6:
                passes = ([(0, h) for h in range(4)] + [(2, h) for h in range(4)]
                          + [(3, h) for h in range(4)] + [(1, h) for h in range(4)])
            elif om == 7:
                passes = ([(0, h) for h in range(4)] + [(3, h) for h in range(4)]
                          + [(1, h) for h in range(4)] + [(2, h) for h in range(4)])
            elif om == 8:
                passes = ([(1, h) for h in range(4)] + [(0, h) for h in range(4)]
                          + [(3, h) for h in range(4)] + [(2, h) for h in range(4)])
            else:
                passes = [(qc, hp) for qc in range(_QC) for hp in range(_FT)]

            # chunk 0's projections were emitted upfront
            task_done = {("v", sb) for sb in range(4)}
            task_done |= {("qk", 0, h2, w) for h2 in range(_FT) for w in range(2)}

            def run_task(t):
                if t in task_done:
                    return False
                task_done.add(t)
                if t[0] == "v":
                    proj_v(t[1])
                else:
                    proj_qk1(t[1], t[2], t[3])
                return True

            def force_deps(qc, hp):
                # V of all chunks up to qc (AV reads them) and this pass's
                # own Q/K projection
                for sb in range(4 * qc + 4):
                    run_task(("v", sb))
                run_task(("qk", qc, hp, 0))
                run_task(("qk", qc, hp, 1))

            # background drip order: chunk-major, V before QK
            bg = [("v", sb) for sc in range(1, _QC)
                  for sb in range(4 * sc, 4 * sc + 4)]
            bg += [("qk", sc, h2, w) for sc in range(1, _QC)
                   for h2 in range(_FT) for w in range(2)]

            final_pass[0] = passes[-1]
            units_left = sum(2 * q + 2 for q, _ in passes)
            for qc, hp in passes:
                if qc >= 1:
                    force_deps(qc, hp)
                for kbg in range(2 * qc + 2):
                    while bg and bg[0] in task_done:
                        bg.pop(0)
                    if bg and len(bg) * 2 >= units_left:
                        run_task(bg.pop(0))
                    units_left -= 1
                    if (soft and not bg
                            and kb_ctr % opts.get("soft_rate", 8) == 0):
                        oproj_qb(soft.pop(0))
                    kb_ctr += 1
                    inflight.append((qc, hp, kbg, emit_scores(qc, hp, kbg)))
                    if len(inflight) >= opts.get("depth", 9):
                        q0, h0, k0, e0 = inflight.pop(0)
                        finish_unit(q0, h0, k0, e0)
            for q0, h0, k0, e0 in inflight:
                finish_unit(q0, h0, k0, e0)

            if opts.get("dbg"):
                ydbg = nc.dram_tensor("ctx_dbg", [128, _FT, _S], bf16,
                                      kind="ExternalOutput")
                nc.sync.dma_start(ydbg[:], ctxm[:])
                qdbg = nc.dram_tensor("q_dbg", [128, _FT, _S], bf16,
                                      kind="ExternalOutput")
                for ft in range(_FT):
                    nc.sync.dma_start(qdbg[:][:, ft, :], qts[ft][:])
                kdbg = nc.dram_tensor("k_dbg", [128, _FT, _S], bf16,
                                      kind="ExternalOutput")
                for ft in range(_FT):
                    nc.sync.dma_start(kdbg[:][:, ft, :], kts[ft][:])
                vdbg = nc.dram_tensor("v_dbg", [128, _SB, _HPC, 65], bf16,
                                      kind="ExternalOutput")
                nc.sync.dma_start(vdbg[:], vxs[:])

    nc.compile()
    return nc


def _get_nc(opts=None):
    key = tuple(sorted((opts or {}).items()))
    if key not in _cache:
        _cache[key] = _build_nc(opts)
    return _cache[key]


def _shard(x, Wq, Wk, Wv, Wo):
    in_maps = []
    for c in range(_NCORES):
        b, hh = divmod(c, 2)
        cols = slice(512 * hh, 512 * hh + 512)
        in_maps.append({
            "xt": np.ascontiguousarray(x[b].T).astype(_BF16),
            "wq": (Wq[:, cols] * np.float32(0.125)).astype(_BF16),
            "wk": np.ascontiguousarray(Wk[:, cols]).astype(_BF16),
            "wv": np.ascontiguousarray(Wv[:, cols]).astype(_BF16),
            "wo": np.ascontiguousarray(Wo[cols, :]).astype(_BF16),
        })
    return in_maps


def _run(inputs, trace=False):
    from concourse import bass_utils

    x = np.asarray(inputs["x"], dtype=np.float32)
    Wq = np.asarray(inputs["Wq"], dtype=np.float32)
    Wk = np.asarray(inputs["Wk"], dtype=np.float32)
    Wv = np.asarray(inputs["Wv"], dtype=np.float32)
    Wo = np.asarray(inputs["Wo"], dtype=np.float32)
    # attention_mask is all-ones by problem spec (fill=ones) -> no-op.

    nc = _get_nc()
    res = bass_utils.run_bass_kernel_spmd(
        nc, _shard(x, Wq, Wk, Wv, Wo), core_ids=list(range(_NCORES)), trace=trace
    )
    ys = [np.asarray(r["y"]).astype(np.float32) for r in res.results]
    out = np.stack([ys[2 * b] + ys[2 * b + 1] for b in range(_B)])
    return out, res


def kernel(**inputs):
    return _run(inputs)[0]
